# revision 1
# baseline (speedup 1.0000x reference)
"""Bounding-box discipline penalty kernel for Trainium2 (8 NeuronCores).

Reference computation:
    pred_mask = max_c(prediction_probs) > 0.3   [B, H, W]
    true_mask = max_c(expected_onehot)  > 0.5   [B, H, W]
    per-sample bboxes from the masks -> area/center penalties -> scalar mean.

Strategy (pure data parallel, B=16 over 8 cores => 2 samples/core):
  * Device: stream both tensors' shards through SBUF and compute the
    per-pixel channel max, laid out as pixmax[partition=128, 512] per
    (tensor, sample). That is the entire memory-bound part (reads 128 MiB
    per core at HBM line rate; the reduction overlaps the DMA stream).
    The last sample-tensor's chunks taper off in size and alternate
    between the Vector and GpSimd engines so the final reduction drains
    in parallel instead of serializing after the last DMA.
  * Host: fold the tiny [4, 128, 512] per-core results into per-sample
    row/col maxima (exact max operations, order-independent), then do the
    O(B) bbox + penalty math exactly as the reference does.

Self-contained: hardcodes shapes from the problem spec.
"""

import numpy as np

THRESHOLD = 0.3
PENALTY_WEIGHT = 0.05

B, H, W, C = 16, 256, 256, 128
N_CORES = 8
SPC = B // N_CORES            # samples per core = 2
NST = 2 * SPC                 # sample-tensor streams per core = 4
PIX = H * W                   # 65536 pixels per sample
NPART = 128
PPP = PIX // NPART            # 512 pixels per partition
EPP = PPP * C                 # 65536 f32 elems per partition per sample
NT = 4                        # full-size tiles per sample-tensor
F = EPP // NT                 # 16384 elems/partition per DMA (8 MiB tiles)
NB = 3                        # SBUF load-buffer ring depth

_cache = {}


def _chunk_schedule():
    """Load plan: list of (st, elem offset, size, slot, slot offset).

    st 0..2 stream as uniform 8 MiB chunks round-robin over the three
    16384-elem SBUF slots. The last sample-tensor keeps only two 8 MiB
    chunks and then tapers (3x8192, 4096, 2048, 1024, 2x512) packed into
    sub-regions of the slots, so the final DVE reduces are short and the
    taper DMAs are gated only on long-finished reduces.
    """
    plan = []
    k = 0
    for st in range(NST - 1):
        for i in range(NT):
            plan.append((st, i * F, F, k % 3, 0))
            k += 1
    st = NST - 1
    tail_sizes = [F, F, F, F // 2, F // 4, F // 8, F // 16, F // 16]
    assert sum(tail_sizes) == EPP
    placements = [
        (k % 3, 0),
        ((k + 1) % 3, 0),
        ((k + 2) % 3, 0),
        (k % 3, 0),
        (k % 3, F // 2),
        (k % 3, 3 * F // 4),
        (k % 3, 7 * F // 8),
        (k % 3, 15 * F // 16),
    ]
    off = 0
    for sz, (slot, soff) in zip(tail_sizes, placements):
        plan.append((st, off, sz, slot, soff))
        off += sz
    return plan


def _build_nc():
    from contextlib import ExitStack

    import concourse.bass as bass
    import concourse.mybir as mybir

    f32 = mybir.dt.float32
    nc = bass.Bass()
    pred = nc.dram_tensor("pred", [SPC, NPART, EPP], f32, kind="ExternalInput")
    tru = nc.dram_tensor("tru", [SPC, NPART, EPP], f32, kind="ExternalInput")
    # pixmax per sample-tensor: [st, partition, pixel-in-partition]
    outp = nc.dram_tensor("outp", [NST, NPART, PPP], f32, kind="ExternalOutput")

    srcs = [(pred, 0), (pred, 1), (tru, 0), (tru, 1)]
    plan = _chunk_schedule()
    nloads = len(plan)

    # gate[k]: 1-based reduce count that must be reached before load k may
    # overwrite its slot region (latest earlier load overlapping the region)
    gate = []
    for k, (_st, _off, _sz, slot, soff) in enumerate(plan):
        g = 0
        for j in range(k):
            _stj, _offj, szj, slotj, soffj = plan[j]
            if slotj == slot and soffj < soff + plan[k][2] and soff < soffj + szj:
                g = j + 1
        gate.append(g)
    # last load index per st (reduces complete in load order)
    last_of_st = {}
    for k, (st, _o, _s, _sl, _so) in enumerate(plan):
        last_of_st[st] = k

    with ExitStack() as ctx:
        buf = [
            ctx.enter_context(nc.sbuf_tensor(f"buf{i}", [NPART, F], f32))
            for i in range(NB)
        ]
        pm = [
            ctx.enter_context(nc.sbuf_tensor(f"pm{i}", [NPART, PPP], f32))
            for i in range(NST)
        ]
        lsems = [
            ctx.enter_context(nc.semaphore(f"ls{i}")) for i in range(nloads)
        ]
        vfree = ctx.enter_context(nc.semaphore("vfree"))
        dummy = ctx.enter_context(nc.semaphore("dummy"))
        outsem = ctx.enter_context(nc.semaphore("outsem"))
        block = ctx.enter_context(nc.Block())

        @block.sync
        def _(sync):
            for k, (st, off, sz, slot, soff) in enumerate(plan):
                src, s = srcs[st]
                if gate[k]:
                    sync.wait_ge(vfree, gate[k])
                sync.dma_start(
                    out=buf[slot][:, soff : soff + sz],
                    in_=src[s, :, off : off + sz],
                ).then_inc(lsems[k], 16)

        @block.vector
        def _(vector):
            for k, (st, off, sz, slot, soff) in enumerate(plan):
                vector.wait_ge(lsems[k], 16)
                vector.reduce_max(
                    out=pm[st][:, off // C : (off + sz) // C],
                    in_=buf[slot][:, soff : soff + sz].rearrange(
                        "p (a c) -> p a c", c=C
                    ),
                    axis=mybir.AxisListType.X,
                ).then_inc(vfree, 1)

        @block.scalar
        def _(scalar):
            n_outs = 0

            def flush(st, px_lo, px_hi, need_v):
                scalar.wait_ge(vfree, need_v)
                scalar.dma_start(
                    out=outp[st, :, px_lo:px_hi],
                    in_=pm[st][:, px_lo:px_hi],
                ).then_inc(outsem, 16)

            for st in range(NST):
                if st < NST - 1:
                    flush(st, 0, PPP, last_of_st[st] + 1)
                    n_outs += 1
                else:
                    # tapered st: flush the big chunks' pixels early, then
                    # the tapered remainder once everything is reduced
                    sizes = [p[2] for p in plan if p[0] == st]
                    nbig = sum(1 for s_ in sizes if s_ == F)
                    head_px = nbig * F // C
                    first = nloads - len(sizes)
                    flush(st, 0, head_px, first + nbig)
                    flush(st, head_px, PPP, last_of_st[st] + 1)
                    n_outs += 2
            scalar.wait_ge(outsem, 16 * n_outs)

    return nc


def _run_device(pred_np, true_np, trace=False):
    from concourse.bass_utils import run_bass_kernel_spmd

    if "nc" not in _cache:
        _cache["nc"] = _build_nc()
    nc = _cache["nc"]

    # [B, H, W, C] -> per-core shards [SPC, 128, EPP]
    pred_sh = pred_np.reshape(N_CORES, SPC, NPART, EPP)
    true_sh = true_np.reshape(N_CORES, SPC, NPART, EPP)
    in_maps = [
        {"pred": pred_sh[i], "tru": true_sh[i]} for i in range(N_CORES)
    ]
    res = run_bass_kernel_spmd(
        nc, in_maps, core_ids=list(range(N_CORES)), trace=trace
    )
    # [N_CORES, NST, 128, PPP]
    pms = np.stack([res.results[i]["outp"] for i in range(N_CORES)])
    return pms, res


def _bbox_from_maxes(rowv, colv, thresh):
    """rowv [B,H], colv [B,W] float32 maxima -> bbox coords, matching _bbox."""
    row_any = rowv > thresh
    col_any = colv > thresh
    ys = np.arange(H, dtype=np.float32)
    xs = np.arange(W, dtype=np.float32)
    y_min = np.where(row_any, ys, np.float32(H)).min(axis=1)
    y_max = np.where(row_any, ys, np.float32(-1)).max(axis=1)
    x_min = np.where(col_any, xs, np.float32(W)).min(axis=1)
    x_max = np.where(col_any, xs, np.float32(-1)).max(axis=1)
    empty = ~row_any.any(axis=1)
    f32 = np.float32
    y_min = np.where(empty, f32(0.0), y_min).astype(np.float32)
    x_min = np.where(empty, f32(0.0), x_min).astype(np.float32)
    y_max = np.where(empty, f32(1.0), y_max).astype(np.float32)
    x_max = np.where(empty, f32(1.0), x_max).astype(np.float32)
    return y_min, x_min, y_max, x_max


def _penalty_from_pms(pms):
    """pms [N_CORES, NST, 128, PPP] -> scalar penalty (float32)."""
    # pms[c, st] covers sample 2c + (st % SPC); st//SPC==0 -> pred, ==1 -> true
    pm4 = pms.reshape(N_CORES, 2, SPC, NPART, 2, W)  # [c, tensor, s, p, r, w]
    pm4 = pm4.transpose(1, 0, 2, 3, 4, 5).reshape(2, B, NPART, 2, W)
    rowv = pm4.max(axis=4)            # [2, B, 128, 2] -> rows 2p+r
    rowv = rowv.reshape(2, B, H)
    colv = pm4.max(axis=(2, 3))       # [2, B, W]

    p = _bbox_from_maxes(rowv[0], colv[0], np.float32(THRESHOLD))
    t = _bbox_from_maxes(rowv[1], colv[1], np.float32(0.5))
    py_min, px_min, py_max, px_max = p
    ty_min, tx_min, ty_max, tx_max = t

    one = np.float32(1.0)
    pred_area = (py_max - py_min + one) * (px_max - px_min + one)
    true_area = (ty_max - ty_min + one) * (tx_max - tx_min + one)
    area_penalty = np.maximum(pred_area - true_area, np.float32(0.0)) / (
        true_area + one
    )
    two = np.float32(2.0)
    dy = (py_min + py_max) / two - (ty_min + ty_max) / two
    dx = (px_min + px_max) / two - (tx_min + tx_max) / two
    center_offset = np.sqrt(dy * dy + dx * dx).astype(np.float32) / np.float32(
        20.0
    )
    penalties = area_penalty + center_offset
    return np.float32(PENALTY_WEIGHT) * penalties.mean(dtype=np.float32)


def _run(prediction_probs, expected_onehot, trace=False):
    pred_np = np.ascontiguousarray(
        np.asarray(prediction_probs, dtype=np.float32)
    )
    true_np = np.ascontiguousarray(
        np.asarray(expected_onehot, dtype=np.float32)
    )
    assert pred_np.shape == (B, H, W, C), pred_np.shape
    assert true_np.shape == (B, H, W, C), true_np.shape
    pms, res = _run_device(pred_np, true_np, trace=trace)
    val = _penalty_from_pms(pms)
    return np.asarray(val, dtype=np.float32), res


def kernel(prediction_probs, expected_onehot):
    out, _ = _run(prediction_probs, expected_onehot, trace=False)
    return out



# revision 5
# speedup vs baseline: 21.8906x; 21.8906x over previous
"""Bounding-box discipline penalty kernel for Trainium2 (8 NeuronCores).

Reference computation:
    pred_mask = max_c(prediction_probs) > 0.3   [B, H, W]
    true_mask = max_c(expected_onehot)  > 0.5   [B, H, W]
    per-sample bboxes from the masks -> area/center penalties -> scalar mean.

Key observation: the bboxes only need, per row/col, whether ANY pixel in it
exceeds the threshold. That admits an adaptive algorithm that certifies the
boxes from a tiny probe instead of streaming all 1 GiB:

  * Probe pass (device): for every (sample, tensor) read rows 0 and 255
    (first PROBE_C channels of each pixel) and reduce the per-pixel channel
    max on device. 256 KiB per core instead of 128 MiB.
  * Host: if for every (sample, tensor) row 0 has a hit, row 255 has a hit,
    and every column has a hit in one of the two rows (threshold 0.3 for
    pred, 0.5 for true), then y_min=0, y_max=H-1, x_min=0, x_max=W-1 are
    EXACT — a subset hit implies the full-data `any` is true. Both boxes are
    the full image, so every penalty term is exactly 0 and the result is
    exactly the reference's 0.0.
  * Otherwise fall back to the full-read streaming kernel (below), which is
    exact for arbitrary inputs.

The probe is sound, never approximate: it only concludes when the subset
evidence forces the reference's own answer; anything else takes the full
path.

Self-contained: hardcodes shapes from the problem spec.
"""

import numpy as np

THRESHOLD = 0.3
PENALTY_WEIGHT = 0.05

B, H, W, C = 16, 256, 256, 128
N_CORES = 8
SPC = B // N_CORES            # samples per core = 2
NST = 2 * SPC                 # sample-tensor streams per core = 4
PIX = H * W                   # 65536 pixels per sample
NPART = 128
PPP = PIX // NPART            # 512 pixels per partition
EPP = PPP * C                 # 65536 f32 elems per partition per sample
NT = 4                        # full-size tiles per sample-tensor
F = EPP // NT                 # 16384 elems/partition per DMA (8 MiB tiles)
NB = 3                        # SBUF load-buffer ring depth

# ---- probe pass constants ----
PROBE_C = 32                  # channels shipped per probe pixel
PROBE_ROWS = (0, H - 1)
PROBE_PIX = NST * 2 * W       # probe pixels per core = 2048
PROBE_E = PROBE_PIX * PROBE_C           # probe elems per core = 65536
PROBE_FPP = PROBE_E // NPART            # 512 elems per partition
PROBE_APP = PROBE_FPP // PROBE_C        # 16 pixels per partition

_cache = {}


# --------------------------------------------------------------------------
# Probe pass: tiny device kernel + host-side exact certification
# --------------------------------------------------------------------------

def _build_nc_probe():
    from contextlib import ExitStack

    import concourse.bass as bass
    import concourse.mybir as mybir

    f32 = mybir.dt.float32
    nc = bass.Bass()
    probe = nc.dram_tensor(
        "probe", [NPART, PROBE_FPP], f32, kind="ExternalInput"
    )
    outp = nc.dram_tensor(
        "outp", [NPART, PROBE_APP], f32, kind="ExternalOutput"
    )

    with ExitStack() as ctx:
        buf = ctx.enter_context(
            nc.sbuf_tensor("buf", [NPART, PROBE_FPP], f32)
        )
        pm = ctx.enter_context(nc.sbuf_tensor("pm", [NPART, PROBE_APP], f32))
        ls = ctx.enter_context(nc.semaphore("ls"))
        rs = ctx.enter_context(nc.semaphore("rs"))
        osem = ctx.enter_context(nc.semaphore("osem"))
        block = ctx.enter_context(nc.Block())

        @block.sync
        def _(sync):
            sync.dma_start(out=buf[:, :], in_=probe[:, :]).then_inc(ls, 16)

        @block.vector
        def _(vector):
            vector.wait_ge(ls, 16)
            vector.reduce_max(
                out=pm[:, :],
                in_=buf[:, :].rearrange("p (a c) -> p a c", c=PROBE_C),
                axis=mybir.AxisListType.X,
            ).then_inc(rs, 1)

        @block.scalar
        def _(scalar):
            scalar.wait_ge(rs, 1)
            scalar.dma_start(out=outp[:, :], in_=pm[:, :]).then_inc(osem, 16)
            scalar.wait_ge(osem, 16)

    return nc


def _probe_shards(pred_np, true_np):
    """Per-core probe arrays [NPART, PROBE_FPP].

    Element order per core: (st, row, w, c) with st = [pred s0, pred s1,
    true s0, true s1], rows (0, H-1), c < PROBE_C.
    """
    pr = pred_np[:, PROBE_ROWS, :, :PROBE_C]  # [B, 2, W, PROBE_C]
    tr = true_np[:, PROBE_ROWS, :, :PROBE_C]
    pr = pr.reshape(N_CORES, SPC, 2, W, PROBE_C)
    tr = tr.reshape(N_CORES, SPC, 2, W, PROBE_C)
    probe = np.concatenate([pr, tr], axis=1)  # [N_CORES, NST, 2, W, PROBE_C]
    return np.ascontiguousarray(probe).reshape(N_CORES, NPART, PROBE_FPP)


def _run_probe(pred_np, true_np, trace=False):
    from concourse.bass_utils import run_bass_kernel_spmd

    if "nc_probe" not in _cache:
        _cache["nc_probe"] = _build_nc_probe()
    nc = _cache["nc_probe"]

    shards = _probe_shards(pred_np, true_np)
    in_maps = [{"probe": shards[i]} for i in range(N_CORES)]
    res = run_bass_kernel_spmd(
        nc, in_maps, core_ids=list(range(N_CORES)), trace=trace
    )
    # [N_CORES, NPART, PROBE_APP] -> per-pixel maxima [N_CORES, NST, 2, W]
    pms = np.stack([res.results[i]["outp"] for i in range(N_CORES)])
    vals = pms.reshape(N_CORES, NST, 2, W)
    return vals, res


def _probe_certifies(vals):
    """True iff the probe maxima force full-image boxes for every sample.

    vals: [N_CORES, NST, 2, W] per-pixel channel maxima of rows 0 and H-1.
    """
    pred_v = vals[:, :SPC]          # [N_CORES, SPC, 2, W]
    true_v = vals[:, SPC:]
    for v, thr in ((pred_v, THRESHOLD), (true_v, 0.5)):
        hit = v > np.float32(thr)
        if not hit.any(axis=3).all():          # each probed row has a hit
            return False
        if not hit.any(axis=2).all():          # each col hit in row 0 or H-1
            return False
    return True


# --------------------------------------------------------------------------
# Full-read fallback: exact for arbitrary inputs (streams all 128 MiB/core)
# --------------------------------------------------------------------------

def _chunk_schedule():
    """Load plan: list of (st, elem offset, size, slot, slot offset).

    st 0..2 stream as uniform 8 MiB chunks round-robin over the three
    16384-elem SBUF slots. The last sample-tensor keeps only two 8 MiB
    chunks and then tapers (3x8192, 4096, 2048, 1024, 2x512) packed into
    sub-regions of the slots, so the final DVE reduces are short and the
    taper DMAs are gated only on long-finished reduces.
    """
    plan = []
    k = 0
    for st in range(NST - 1):
        for i in range(NT):
            plan.append((st, i * F, F, k % 3, 0))
            k += 1
    st = NST - 1
    tail_sizes = [F, F, F, F // 2, F // 4, F // 8, F // 16, F // 16]
    assert sum(tail_sizes) == EPP
    placements = [
        (k % 3, 0),
        ((k + 1) % 3, 0),
        ((k + 2) % 3, 0),
        (k % 3, 0),
        (k % 3, F // 2),
        (k % 3, 3 * F // 4),
        (k % 3, 7 * F // 8),
        (k % 3, 15 * F // 16),
    ]
    off = 0
    for sz, (slot, soff) in zip(tail_sizes, placements):
        plan.append((st, off, sz, slot, soff))
        off += sz
    return plan


def _build_nc():
    from contextlib import ExitStack

    import concourse.bass as bass
    import concourse.mybir as mybir

    f32 = mybir.dt.float32
    nc = bass.Bass()
    pred = nc.dram_tensor("pred", [SPC, NPART, EPP], f32, kind="ExternalInput")
    tru = nc.dram_tensor("tru", [SPC, NPART, EPP], f32, kind="ExternalInput")
    # pixmax per sample-tensor: [st, partition, pixel-in-partition]
    outp = nc.dram_tensor("outp", [NST, NPART, PPP], f32, kind="ExternalOutput")

    srcs = [(pred, 0), (pred, 1), (tru, 0), (tru, 1)]
    plan = _chunk_schedule()
    nloads = len(plan)

    # gate[k]: 1-based reduce count that must be reached before load k may
    # overwrite its slot region (latest earlier load overlapping the region)
    gate = []
    for k, (_st, _off, _sz, slot, soff) in enumerate(plan):
        g = 0
        for j in range(k):
            _stj, _offj, szj, slotj, soffj = plan[j]
            if slotj == slot and soffj < soff + plan[k][2] and soff < soffj + szj:
                g = j + 1
        gate.append(g)
    # last load index per st (reduces complete in load order)
    last_of_st = {}
    for k, (st, _o, _s, _sl, _so) in enumerate(plan):
        last_of_st[st] = k

    with ExitStack() as ctx:
        buf = [
            ctx.enter_context(nc.sbuf_tensor(f"buf{i}", [NPART, F], f32))
            for i in range(NB)
        ]
        pm = [
            ctx.enter_context(nc.sbuf_tensor(f"pm{i}", [NPART, PPP], f32))
            for i in range(NST)
        ]
        lsems = [
            ctx.enter_context(nc.semaphore(f"ls{i}")) for i in range(nloads)
        ]
        vfree = ctx.enter_context(nc.semaphore("vfree"))
        dummy = ctx.enter_context(nc.semaphore("dummy"))
        outsem = ctx.enter_context(nc.semaphore("outsem"))
        block = ctx.enter_context(nc.Block())

        @block.sync
        def _(sync):
            for k, (st, off, sz, slot, soff) in enumerate(plan):
                src, s = srcs[st]
                if gate[k]:
                    sync.wait_ge(vfree, gate[k])
                sync.dma_start(
                    out=buf[slot][:, soff : soff + sz],
                    in_=src[s, :, off : off + sz],
                ).then_inc(lsems[k], 16)

        @block.vector
        def _(vector):
            for k, (st, off, sz, slot, soff) in enumerate(plan):
                vector.wait_ge(lsems[k], 16)
                vector.reduce_max(
                    out=pm[st][:, off // C : (off + sz) // C],
                    in_=buf[slot][:, soff : soff + sz].rearrange(
                        "p (a c) -> p a c", c=C
                    ),
                    axis=mybir.AxisListType.X,
                ).then_inc(vfree, 1)

        @block.scalar
        def _(scalar):
            n_outs = 0

            def flush(st, px_lo, px_hi, need_v):
                scalar.wait_ge(vfree, need_v)
                scalar.dma_start(
                    out=outp[st, :, px_lo:px_hi],
                    in_=pm[st][:, px_lo:px_hi],
                ).then_inc(outsem, 16)

            for st in range(NST):
                if st < NST - 1:
                    flush(st, 0, PPP, last_of_st[st] + 1)
                    n_outs += 1
                else:
                    # tapered st: flush the big chunks' pixels early, then
                    # the tapered remainder once everything is reduced
                    sizes = [p[2] for p in plan if p[0] == st]
                    nbig = sum(1 for s_ in sizes if s_ == F)
                    head_px = nbig * F // C
                    first = nloads - len(sizes)
                    flush(st, 0, head_px, first + nbig)
                    flush(st, head_px, PPP, last_of_st[st] + 1)
                    n_outs += 2
            scalar.wait_ge(outsem, 16 * n_outs)

    return nc


def _run_device(pred_np, true_np, trace=False):
    from concourse.bass_utils import run_bass_kernel_spmd

    if "nc" not in _cache:
        _cache["nc"] = _build_nc()
    nc = _cache["nc"]

    # [B, H, W, C] -> per-core shards [SPC, 128, EPP]
    pred_sh = pred_np.reshape(N_CORES, SPC, NPART, EPP)
    true_sh = true_np.reshape(N_CORES, SPC, NPART, EPP)
    in_maps = [
        {"pred": pred_sh[i], "tru": true_sh[i]} for i in range(N_CORES)
    ]
    res = run_bass_kernel_spmd(
        nc, in_maps, core_ids=list(range(N_CORES)), trace=trace
    )
    # [N_CORES, NST, 128, PPP]
    pms = np.stack([res.results[i]["outp"] for i in range(N_CORES)])
    return pms, res


def _bbox_from_maxes(rowv, colv, thresh):
    """rowv [B,H], colv [B,W] float32 maxima -> bbox coords, matching _bbox."""
    row_any = rowv > thresh
    col_any = colv > thresh
    ys = np.arange(H, dtype=np.float32)
    xs = np.arange(W, dtype=np.float32)
    y_min = np.where(row_any, ys, np.float32(H)).min(axis=1)
    y_max = np.where(row_any, ys, np.float32(-1)).max(axis=1)
    x_min = np.where(col_any, xs, np.float32(W)).min(axis=1)
    x_max = np.where(col_any, xs, np.float32(-1)).max(axis=1)
    empty = ~row_any.any(axis=1)
    f32 = np.float32
    y_min = np.where(empty, f32(0.0), y_min).astype(np.float32)
    x_min = np.where(empty, f32(0.0), x_min).astype(np.float32)
    y_max = np.where(empty, f32(1.0), y_max).astype(np.float32)
    x_max = np.where(empty, f32(1.0), x_max).astype(np.float32)
    return y_min, x_min, y_max, x_max


def _penalty_from_pms(pms):
    """pms [N_CORES, NST, 128, PPP] -> scalar penalty (float32)."""
    # pms[c, st] covers sample 2c + (st % SPC); st//SPC==0 -> pred, ==1 -> true
    pm4 = pms.reshape(N_CORES, 2, SPC, NPART, 2, W)  # [c, tensor, s, p, r, w]
    pm4 = pm4.transpose(1, 0, 2, 3, 4, 5).reshape(2, B, NPART, 2, W)
    rowv = pm4.max(axis=4)            # [2, B, 128, 2] -> rows 2p+r
    rowv = rowv.reshape(2, B, H)
    colv = pm4.max(axis=(2, 3))       # [2, B, W]

    p = _bbox_from_maxes(rowv[0], colv[0], np.float32(THRESHOLD))
    t = _bbox_from_maxes(rowv[1], colv[1], np.float32(0.5))
    py_min, px_min, py_max, px_max = p
    ty_min, tx_min, ty_max, tx_max = t

    one = np.float32(1.0)
    pred_area = (py_max - py_min + one) * (px_max - px_min + one)
    true_area = (ty_max - ty_min + one) * (tx_max - tx_min + one)
    area_penalty = np.maximum(pred_area - true_area, np.float32(0.0)) / (
        true_area + one
    )
    two = np.float32(2.0)
    dy = (py_min + py_max) / two - (ty_min + ty_max) / two
    dx = (px_min + px_max) / two - (tx_min + tx_max) / two
    center_offset = np.sqrt(dy * dy + dx * dx).astype(np.float32) / np.float32(
        20.0
    )
    penalties = area_penalty + center_offset
    return np.float32(PENALTY_WEIGHT) * penalties.mean(dtype=np.float32)


# --------------------------------------------------------------------------
# Entry points
# --------------------------------------------------------------------------

def _run(prediction_probs, expected_onehot, trace=False):
    """Returns (value, list of device results)."""
    pred_np = np.ascontiguousarray(
        np.asarray(prediction_probs, dtype=np.float32)
    )
    true_np = np.ascontiguousarray(
        np.asarray(expected_onehot, dtype=np.float32)
    )
    assert pred_np.shape == (B, H, W, C), pred_np.shape
    assert true_np.shape == (B, H, W, C), true_np.shape

    vals, res = _run_probe(pred_np, true_np, trace=trace)
    if _probe_certifies(vals):
        # Boxes are exactly the full image for every (sample, tensor):
        # pred_area == true_area and centers coincide, so every penalty
        # term is exactly 0.0, matching the reference bit-for-bit.
        return np.asarray(np.float32(0.0)), [res]

    pms, res_full = _run_device(pred_np, true_np, trace=trace)
    val = _penalty_from_pms(pms)
    return np.asarray(val, dtype=np.float32), [res, res_full]


def kernel(prediction_probs, expected_onehot):
    out, _ = _run(prediction_probs, expected_onehot, trace=False)
    return out


# revision 7
# speedup vs baseline: 24.2924x; 1.1097x over previous
"""Bounding-box discipline penalty kernel for Trainium2 (8 NeuronCores).

Reference computation:
    pred_mask = max_c(prediction_probs) > 0.3   [B, H, W]
    true_mask = max_c(expected_onehot)  > 0.5   [B, H, W]
    per-sample bboxes from the masks -> area/center penalties -> scalar mean.

Key observation: the bboxes only need, per row/col, whether ANY pixel in it
exceeds the threshold. That admits an adaptive algorithm that certifies the
boxes from a tiny probe instead of streaming all 1 GiB:

  * Probe pass (device): for every (sample, tensor) read rows 0 and 255
    (first PROBE_C channels of each pixel) and reduce the per-pixel channel
    max on device. 256 KiB per core instead of 128 MiB.
  * Host: if for every (sample, tensor) row 0 has a hit, row 255 has a hit,
    and every column has a hit in one of the two rows (threshold 0.3 for
    pred, 0.5 for true), then y_min=0, y_max=H-1, x_min=0, x_max=W-1 are
    EXACT — a subset hit implies the full-data `any` is true. Both boxes are
    the full image, so every penalty term is exactly 0 and the result is
    exactly the reference's 0.0.
  * Otherwise fall back to the full-read streaming kernel (below), which is
    exact for arbitrary inputs.

The probe is sound, never approximate: it only concludes when the subset
evidence forces the reference's own answer; anything else takes the full
path.

Self-contained: hardcodes shapes from the problem spec.
"""

import numpy as np

THRESHOLD = 0.3
PENALTY_WEIGHT = 0.05

B, H, W, C = 16, 256, 256, 128
N_CORES = 8
SPC = B // N_CORES            # samples per core = 2
NST = 2 * SPC                 # sample-tensor streams per core = 4
PIX = H * W                   # 65536 pixels per sample
NPART = 128
PPP = PIX // NPART            # 512 pixels per partition
EPP = PPP * C                 # 65536 f32 elems per partition per sample
NT = 4                        # full-size tiles per sample-tensor
F = EPP // NT                 # 16384 elems/partition per DMA (8 MiB tiles)
NB = 3                        # SBUF load-buffer ring depth

# ---- probe pass constants ----
PROBE_C = 16                  # channels shipped per probe pixel
PROBE_ROWS = (0, H - 1)
PROBE_PIX = NST * 2 * W       # probe pixels per core = 2048
PROBE_E = PROBE_PIX * PROBE_C           # probe elems per core = 32768
PROBE_FPP = PROBE_E // NPART            # 256 elems per partition
PROBE_APP = PROBE_FPP // PROBE_C        # 16 pixels per partition

_cache = {}


# --------------------------------------------------------------------------
# Probe pass: tiny device kernel + host-side exact certification
# --------------------------------------------------------------------------

def _build_nc_probe():
    from contextlib import ExitStack

    import concourse.bass as bass
    import concourse.mybir as mybir

    f32 = mybir.dt.float32
    nc = bass.Bass(enable_partition_id=False, monotonic_sem_count=0)
    probe = nc.dram_tensor(
        "probe", [NPART, PROBE_FPP], f32, kind="ExternalInput"
    )
    outp = nc.dram_tensor(
        "outp", [NPART, PROBE_APP], f32, kind="ExternalOutput"
    )
    HALF = PROBE_FPP // 2
    HA = PROBE_APP // 2

    with ExitStack() as ctx:
        buf = ctx.enter_context(
            nc.sbuf_tensor("buf", [NPART, PROBE_FPP], f32)
        )
        pm = ctx.enter_context(nc.sbuf_tensor("pm", [NPART, PROBE_APP], f32))
        ls = ctx.enter_context(nc.semaphore("ls"))
        rs = ctx.enter_context(nc.semaphore("rs"))
        osem = ctx.enter_context(nc.semaphore("osem"))
        block = ctx.enter_context(nc.Block())

        @block.sync
        def _(sync):
            sync.dma_start(out=buf[:, :HALF], in_=probe[:, :HALF]).then_inc(
                ls, 16
            )
            sync.dma_start(out=buf[:, HALF:], in_=probe[:, HALF:]).then_inc(
                ls, 16
            )
            sync.wait_ge(rs, 2)
            sync.dma_start(out=outp[:, :], in_=pm[:, :]).then_inc(osem, 16)
            sync.wait_ge(osem, 16)

        @block.vector
        def _(vector):
            vector.wait_ge(ls, 16)
            vector.reduce_max(
                out=pm[:, :HA],
                in_=buf[:, :HALF].rearrange("p (a c) -> p a c", c=PROBE_C),
                axis=mybir.AxisListType.X,
            ).then_inc(rs, 1)
            vector.wait_ge(ls, 32)
            vector.reduce_max(
                out=pm[:, HA:],
                in_=buf[:, HALF:].rearrange("p (a c) -> p a c", c=PROBE_C),
                axis=mybir.AxisListType.X,
            ).then_inc(rs, 1)

    return nc


def _probe_shards(pred_np, true_np):
    """Per-core probe arrays [NPART, PROBE_FPP].

    Element order per core: (st, row, w, c) with st = [pred s0, pred s1,
    true s0, true s1], rows (0, H-1), c < PROBE_C.
    """
    pr = pred_np[:, PROBE_ROWS, :, :PROBE_C]  # [B, 2, W, PROBE_C]
    tr = true_np[:, PROBE_ROWS, :, :PROBE_C]
    pr = pr.reshape(N_CORES, SPC, 2, W, PROBE_C)
    tr = tr.reshape(N_CORES, SPC, 2, W, PROBE_C)
    probe = np.concatenate([pr, tr], axis=1)  # [N_CORES, NST, 2, W, PROBE_C]
    return np.ascontiguousarray(probe).reshape(N_CORES, NPART, PROBE_FPP)


def _run_probe(pred_np, true_np, trace=False):
    from concourse.bass_utils import run_bass_kernel_spmd

    if "nc_probe" not in _cache:
        _cache["nc_probe"] = _build_nc_probe()
    nc = _cache["nc_probe"]

    shards = _probe_shards(pred_np, true_np)
    in_maps = [{"probe": shards[i]} for i in range(N_CORES)]
    res = run_bass_kernel_spmd(
        nc, in_maps, core_ids=list(range(N_CORES)), trace=trace
    )
    # [N_CORES, NPART, PROBE_APP] -> per-pixel maxima [N_CORES, NST, 2, W]
    pms = np.stack([res.results[i]["outp"] for i in range(N_CORES)])
    vals = pms.reshape(N_CORES, NST, 2, W)
    return vals, res


def _probe_certifies(vals):
    """True iff the probe maxima force full-image boxes for every sample.

    vals: [N_CORES, NST, 2, W] per-pixel channel maxima of rows 0 and H-1.
    """
    pred_v = vals[:, :SPC]          # [N_CORES, SPC, 2, W]
    true_v = vals[:, SPC:]
    for v, thr in ((pred_v, THRESHOLD), (true_v, 0.5)):
        hit = v > np.float32(thr)
        if not hit.any(axis=3).all():          # each probed row has a hit
            return False
        if not hit.any(axis=2).all():          # each col hit in row 0 or H-1
            return False
    return True


# --------------------------------------------------------------------------
# Full-read fallback: exact for arbitrary inputs (streams all 128 MiB/core)
# --------------------------------------------------------------------------

def _chunk_schedule():
    """Load plan: list of (st, elem offset, size, slot, slot offset).

    st 0..2 stream as uniform 8 MiB chunks round-robin over the three
    16384-elem SBUF slots. The last sample-tensor keeps only two 8 MiB
    chunks and then tapers (3x8192, 4096, 2048, 1024, 2x512) packed into
    sub-regions of the slots, so the final DVE reduces are short and the
    taper DMAs are gated only on long-finished reduces.
    """
    plan = []
    k = 0
    for st in range(NST - 1):
        for i in range(NT):
            plan.append((st, i * F, F, k % 3, 0))
            k += 1
    st = NST - 1
    tail_sizes = [F, F, F, F // 2, F // 4, F // 8, F // 16, F // 16]
    assert sum(tail_sizes) == EPP
    placements = [
        (k % 3, 0),
        ((k + 1) % 3, 0),
        ((k + 2) % 3, 0),
        (k % 3, 0),
        (k % 3, F // 2),
        (k % 3, 3 * F // 4),
        (k % 3, 7 * F // 8),
        (k % 3, 15 * F // 16),
    ]
    off = 0
    for sz, (slot, soff) in zip(tail_sizes, placements):
        plan.append((st, off, sz, slot, soff))
        off += sz
    return plan


def _build_nc():
    from contextlib import ExitStack

    import concourse.bass as bass
    import concourse.mybir as mybir

    f32 = mybir.dt.float32
    nc = bass.Bass()
    pred = nc.dram_tensor("pred", [SPC, NPART, EPP], f32, kind="ExternalInput")
    tru = nc.dram_tensor("tru", [SPC, NPART, EPP], f32, kind="ExternalInput")
    # pixmax per sample-tensor: [st, partition, pixel-in-partition]
    outp = nc.dram_tensor("outp", [NST, NPART, PPP], f32, kind="ExternalOutput")

    srcs = [(pred, 0), (pred, 1), (tru, 0), (tru, 1)]
    plan = _chunk_schedule()
    nloads = len(plan)

    # gate[k]: 1-based reduce count that must be reached before load k may
    # overwrite its slot region (latest earlier load overlapping the region)
    gate = []
    for k, (_st, _off, _sz, slot, soff) in enumerate(plan):
        g = 0
        for j in range(k):
            _stj, _offj, szj, slotj, soffj = plan[j]
            if slotj == slot and soffj < soff + plan[k][2] and soff < soffj + szj:
                g = j + 1
        gate.append(g)
    # last load index per st (reduces complete in load order)
    last_of_st = {}
    for k, (st, _o, _s, _sl, _so) in enumerate(plan):
        last_of_st[st] = k

    with ExitStack() as ctx:
        buf = [
            ctx.enter_context(nc.sbuf_tensor(f"buf{i}", [NPART, F], f32))
            for i in range(NB)
        ]
        pm = [
            ctx.enter_context(nc.sbuf_tensor(f"pm{i}", [NPART, PPP], f32))
            for i in range(NST)
        ]
        lsems = [
            ctx.enter_context(nc.semaphore(f"ls{i}")) for i in range(nloads)
        ]
        vfree = ctx.enter_context(nc.semaphore("vfree"))
        dummy = ctx.enter_context(nc.semaphore("dummy"))
        outsem = ctx.enter_context(nc.semaphore("outsem"))
        block = ctx.enter_context(nc.Block())

        @block.sync
        def _(sync):
            for k, (st, off, sz, slot, soff) in enumerate(plan):
                src, s = srcs[st]
                if gate[k]:
                    sync.wait_ge(vfree, gate[k])
                sync.dma_start(
                    out=buf[slot][:, soff : soff + sz],
                    in_=src[s, :, off : off + sz],
                ).then_inc(lsems[k], 16)

        @block.vector
        def _(vector):
            for k, (st, off, sz, slot, soff) in enumerate(plan):
                vector.wait_ge(lsems[k], 16)
                vector.reduce_max(
                    out=pm[st][:, off // C : (off + sz) // C],
                    in_=buf[slot][:, soff : soff + sz].rearrange(
                        "p (a c) -> p a c", c=C
                    ),
                    axis=mybir.AxisListType.X,
                ).then_inc(vfree, 1)

        @block.scalar
        def _(scalar):
            n_outs = 0

            def flush(st, px_lo, px_hi, need_v):
                scalar.wait_ge(vfree, need_v)
                scalar.dma_start(
                    out=outp[st, :, px_lo:px_hi],
                    in_=pm[st][:, px_lo:px_hi],
                ).then_inc(outsem, 16)

            for st in range(NST):
                if st < NST - 1:
                    flush(st, 0, PPP, last_of_st[st] + 1)
                    n_outs += 1
                else:
                    # tapered st: flush the big chunks' pixels early, then
                    # the tapered remainder once everything is reduced
                    sizes = [p[2] for p in plan if p[0] == st]
                    nbig = sum(1 for s_ in sizes if s_ == F)
                    head_px = nbig * F // C
                    first = nloads - len(sizes)
                    flush(st, 0, head_px, first + nbig)
                    flush(st, head_px, PPP, last_of_st[st] + 1)
                    n_outs += 2
            scalar.wait_ge(outsem, 16 * n_outs)

    return nc


def _run_device(pred_np, true_np, trace=False):
    from concourse.bass_utils import run_bass_kernel_spmd

    if "nc" not in _cache:
        _cache["nc"] = _build_nc()
    nc = _cache["nc"]

    # [B, H, W, C] -> per-core shards [SPC, 128, EPP]
    pred_sh = pred_np.reshape(N_CORES, SPC, NPART, EPP)
    true_sh = true_np.reshape(N_CORES, SPC, NPART, EPP)
    in_maps = [
        {"pred": pred_sh[i], "tru": true_sh[i]} for i in range(N_CORES)
    ]
    res = run_bass_kernel_spmd(
        nc, in_maps, core_ids=list(range(N_CORES)), trace=trace
    )
    # [N_CORES, NST, 128, PPP]
    pms = np.stack([res.results[i]["outp"] for i in range(N_CORES)])
    return pms, res


def _bbox_from_maxes(rowv, colv, thresh):
    """rowv [B,H], colv [B,W] float32 maxima -> bbox coords, matching _bbox."""
    row_any = rowv > thresh
    col_any = colv > thresh
    ys = np.arange(H, dtype=np.float32)
    xs = np.arange(W, dtype=np.float32)
    y_min = np.where(row_any, ys, np.float32(H)).min(axis=1)
    y_max = np.where(row_any, ys, np.float32(-1)).max(axis=1)
    x_min = np.where(col_any, xs, np.float32(W)).min(axis=1)
    x_max = np.where(col_any, xs, np.float32(-1)).max(axis=1)
    empty = ~row_any.any(axis=1)
    f32 = np.float32
    y_min = np.where(empty, f32(0.0), y_min).astype(np.float32)
    x_min = np.where(empty, f32(0.0), x_min).astype(np.float32)
    y_max = np.where(empty, f32(1.0), y_max).astype(np.float32)
    x_max = np.where(empty, f32(1.0), x_max).astype(np.float32)
    return y_min, x_min, y_max, x_max


def _penalty_from_pms(pms):
    """pms [N_CORES, NST, 128, PPP] -> scalar penalty (float32)."""
    # pms[c, st] covers sample 2c + (st % SPC); st//SPC==0 -> pred, ==1 -> true
    pm4 = pms.reshape(N_CORES, 2, SPC, NPART, 2, W)  # [c, tensor, s, p, r, w]
    pm4 = pm4.transpose(1, 0, 2, 3, 4, 5).reshape(2, B, NPART, 2, W)
    rowv = pm4.max(axis=4)            # [2, B, 128, 2] -> rows 2p+r
    rowv = rowv.reshape(2, B, H)
    colv = pm4.max(axis=(2, 3))       # [2, B, W]

    p = _bbox_from_maxes(rowv[0], colv[0], np.float32(THRESHOLD))
    t = _bbox_from_maxes(rowv[1], colv[1], np.float32(0.5))
    py_min, px_min, py_max, px_max = p
    ty_min, tx_min, ty_max, tx_max = t

    one = np.float32(1.0)
    pred_area = (py_max - py_min + one) * (px_max - px_min + one)
    true_area = (ty_max - ty_min + one) * (tx_max - tx_min + one)
    area_penalty = np.maximum(pred_area - true_area, np.float32(0.0)) / (
        true_area + one
    )
    two = np.float32(2.0)
    dy = (py_min + py_max) / two - (ty_min + ty_max) / two
    dx = (px_min + px_max) / two - (tx_min + tx_max) / two
    center_offset = np.sqrt(dy * dy + dx * dx).astype(np.float32) / np.float32(
        20.0
    )
    penalties = area_penalty + center_offset
    return np.float32(PENALTY_WEIGHT) * penalties.mean(dtype=np.float32)


# --------------------------------------------------------------------------
# Entry points
# --------------------------------------------------------------------------

def _run(prediction_probs, expected_onehot, trace=False):
    """Returns (value, list of device results)."""
    pred_np = np.ascontiguousarray(
        np.asarray(prediction_probs, dtype=np.float32)
    )
    true_np = np.ascontiguousarray(
        np.asarray(expected_onehot, dtype=np.float32)
    )
    assert pred_np.shape == (B, H, W, C), pred_np.shape
    assert true_np.shape == (B, H, W, C), true_np.shape

    vals, res = _run_probe(pred_np, true_np, trace=trace)
    if _probe_certifies(vals):
        # Boxes are exactly the full image for every (sample, tensor):
        # pred_area == true_area and centers coincide, so every penalty
        # term is exactly 0.0, matching the reference bit-for-bit.
        return np.asarray(np.float32(0.0)), [res]

    pms, res_full = _run_device(pred_np, true_np, trace=trace)
    val = _penalty_from_pms(pms)
    return np.asarray(val, dtype=np.float32), [res, res_full]


def kernel(prediction_probs, expected_onehot):
    out, _ = _run(prediction_probs, expected_onehot, trace=False)
    return out
